# revision 12
# baseline (speedup 1.0000x reference)
"""Trainium2 Bass kernel for causal self-attention with GQA + RoPE.

Model: B=2, T=2048, C=2048, H=16 query heads, H_KV=4 kv heads, D=128.

Sharding (8 NeuronCores, pure SPMD, no collectives):
  core i -> batch b = i // 4, kv-group g = i % 4
            (query heads 4g..4g+3, kv head g, all T positions of batch b).
  o_proj uses the row-slice wo[512g:512(g+1), :]; the per-core partial
  [T, C] outputs are summed on the host (the tensor-parallel all-reduce).

v4 (trace-driven): input delivery is HBM-bound (~360GB/s/core shared by all
8 cores; 13MB of packed input needs ~36us) while projections consume x+wq
at that same rate, so the tensor engine stalled ~20us early.  Fix: run the
attention chunks that need no new input BETWEEN projection chunks:

    proj(0) proj(1) attn(0) proj(2) attn(1) proj(3) | attn(2..3)+o_proj

To fit attention PSUM pools (4 banks) alongside projections, each
projection chunk is two 3-output passes (qp0/qp1/kp then qp2/qp3/vp) using
3 PSUM banks + 1 shared aux bank.  Everything else as v3: bf16 operands
(fp32 PSUM), host-packed fat-line DMAs, causal diagonal trimming with a
single [128,128] triangle mask, per-subtile S/exp pipeline, wo resident,
qb-outer o_proj with one 4KB-line output DMA per 128-row block.
"""

import math
import os

import numpy as np

os.environ.setdefault("MYCRO_LOCAL_CACHE", "1")

P = 128
D = 128
H = 16
H_KV = 4
GQ = H // H_KV  # 4 query heads per kv head (= per core)
B = 2
T_FULL = 2048
C_DIM = 2048
NCORES = 8
ROPE_BASE = 10000.0


def _rope_tables(T):
    inv_freq = 1.0 / (ROPE_BASE ** (np.arange(0, D, 2, dtype=np.float32) / D))
    t = np.arange(T, dtype=np.float32)
    freqs = np.outer(t, inv_freq)  # [T, D/2]
    emb = np.concatenate((freqs, freqs), axis=-1)  # [T, D]
    return (
        np.ascontiguousarray(np.cos(emb).T.astype(np.float32)),  # [D, T]
        np.ascontiguousarray(np.sin(emb).T.astype(np.float32)),
    )


def _rot_lhsT():
    # rotate_half(q) = R @ q with R[d, d+64] = -1 (d < 64), R[d, d-64] = +1.
    # matmul computes lhsT.T @ rhs, so pass lhsT = R^T.
    R = np.zeros((D, D), dtype=np.float32)
    half = D // 2
    R[np.arange(half), np.arange(half) + half] = -1.0
    R[np.arange(half) + half, np.arange(half)] = 1.0
    return np.ascontiguousarray(R.T)


def build_nc(T=T_FULL):
    """Build the per-core Bass/Tile program (identical across cores)."""
    from contextlib import ExitStack

    import concourse.mybir as mybir
    import concourse.tile as tile
    from concourse import bacc
    from concourse.masks import make_identity

    f32 = mybir.dt.float32
    f32r = mybir.dt.float32r
    bf16 = mybir.dt.bfloat16
    Exp = mybir.ActivationFunctionType.Exp
    MULT = mybir.AluOpType.mult
    ADD = mybir.AluOpType.add
    SCALE = 1.0 / math.sqrt(D)

    NCC = C_DIM // P  # 16 contraction chunks
    NQC = T // 512  # projection / attention q-chunks (512-wide)
    NCT = C_DIM // 512  # o_proj column tiles
    NKB = T // P  # 128-wide k subtiles

    nc = bacc.Bacc(
        "TRN2",
        target_bir_lowering=False,
        debug=False,
        num_devices=NCORES,
    )

    # host-packed inputs: per-partition-contiguous fat lines
    xq = nc.dram_tensor("xq", [P, NQC * NCC * 512], bf16, kind="ExternalInput").ap()
    wqp = nc.dram_tensor("wqp", [P, NCC * GQ * D], bf16, kind="ExternalInput").ap()
    wkvp = nc.dram_tensor("wkvp", [P, NCC * 2 * D], bf16, kind="ExternalInput").ap()
    wop = nc.dram_tensor("wop", [P, GQ * C_DIM], bf16, kind="ExternalInput").ap()
    cosT = nc.dram_tensor("cosT", [D, T], f32, kind="ExternalInput").ap()
    sinT = nc.dram_tensor("sinT", [D, T], f32, kind="ExternalInput").ap()
    trim = nc.dram_tensor("trim", [P, P], bf16, kind="ExternalInput").ap()
    onesm = nc.dram_tensor("onesm", [P, P], bf16, kind="ExternalInput").ap()
    rotm = nc.dram_tensor("rotm", [P, P], f32r, kind="ExternalInput").ap()
    out = nc.dram_tensor("out", [T, C_DIM], bf16, kind="ExternalOutput").ap()

    with tile.TileContext(nc) as tc, ExitStack() as ctx:
        const = ctx.enter_context(tc.tile_pool(name="const", bufs=1))
        acts = ctx.enter_context(tc.tile_pool(name="acts", bufs=1))

        xq_r = xq.rearrange("p (qc cc t) -> p qc cc t", qc=NQC, cc=NCC)
        wq_r = wqp.rearrange("p (cc n) -> p cc n", cc=NCC)
        wkv_r = wkvp.rearrange("p (cc n) -> p cc n", cc=NCC)
        wo_r = wop.rearrange("p (h c) -> p h c", h=GQ)

        ones_sb = const.tile([P, P], bf16)
        rot_sb = const.tile([P, P], f32r)
        ident = const.tile([P, P], f32)
        tri_sb = const.tile([P, P], bf16)

        # long-lived activations (all bf16)
        qt_sb = [acts.tile([P, T], bf16, name=f"qt{h}") for h in range(GQ)]
        kt_sb = acts.tile([P, T], bf16, name="kt")
        v_sb = acts.tile([P, NKB, D], bf16, name="vnat")
        y_sb = [acts.tile([P, T], bf16, name=f"yt{h}") for h in range(GQ)]
        wo_sb = acts.tile([P, GQ, C_DIM], bf16, name="wo_res")

        # attention pools (outermost so they span both phases)
        with (
            tc.tile_pool(name="pt_pool", bufs=6) as pt_pool,
            tc.tile_pool(name="nrm", bufs=2) as nrm_pool,
            tc.tile_pool(name="s_ps", bufs=2, space="PSUM") as s_ps,
            tc.tile_pool(name="y_ps", bufs=1, space="PSUM") as y_ps,
            tc.tile_pool(name="rs_ps", bufs=1, space="PSUM") as rs_ps,
        ):

            def attn_chunk(aq):
                q0 = aq * 512
                nks = 4 * aq + 4  # 128-wide k subtiles (incl 4 diagonal)
                dstart = nks - 4  # first diagonal subtile index
                for h in range(GQ):
                    yp = y_ps.tile([P, 512], f32, name="yp", tag="yp")
                    rp_ = rs_ps.tile([P, 512], f32, name="rsp", tag="rsp")
                    for ks in range(nks):
                        m = ks - dstart
                        a = 128 * m if m >= 0 else 0
                        sp = s_ps.tile([P, 512], f32, name="sp", tag="sp")
                        pt = pt_pool.tile([P, 512], bf16, name="ptile", tag="pt")
                        nc.tensor.matmul(
                            sp[:, a:512],
                            kt_sb[:, ks * P : (ks + 1) * P],
                            qt_sb[h][:, q0 + a : q0 + 512],
                            start=True,
                            stop=True,
                        )
                        nc.scalar.activation(
                            pt[:, a:512], sp[:, a:512], Exp, scale=SCALE
                        )
                        if m >= 0:  # diagonal: mask the leading 128 cols
                            sl = pt[:, a : a + P]
                            nc.vector.tensor_tensor(sl, sl, tri_sb[:], MULT)
                        first, last = ks == 0, ks == nks - 1
                        nc.tensor.matmul(
                            yp[:, a:512],
                            v_sb[:, ks, :],
                            pt[:, a:512],
                            start=first,
                            stop=last,
                            skip_group_check=True,
                        )
                        nc.tensor.matmul(
                            rp_[:, a:512],
                            ones_sb[:],
                            pt[:, a:512],
                            start=first,
                            stop=last,
                            skip_group_check=True,
                        )
                    # 1/rowsum on DVE (~18 bits, plenty; rowsum >= 1).
                    rinv = nrm_pool.tile([P, 512], f32, name="rinv", tag="rinv")
                    nc.vector.reciprocal_approx_fast(rinv[:], rp_[:])
                    nc.vector.tensor_tensor(
                        y_sb[h][:, q0 : q0 + 512], yp[:], rinv[:], MULT
                    )

            # ---------- phase A: projections interleaved with attn(0..1) ----
            with (
                tc.tile_pool(name="xres", bufs=1) as xres,
                tc.tile_pool(name="pwts", bufs=1) as wpool,
                tc.tile_pool(name="rope_t", bufs=1) as rope_pool,
                tc.tile_pool(name="proj_ps", bufs=1, space="PSUM") as proj_ps,
                tc.tile_pool(name="aux_ps", bufs=1, space="PSUM") as aux_ps,
                tc.tile_pool(name="ptmp", bufs=2) as ptmp,
            ):
                x_sb = xres.tile([P, NQC, NCC, 512], bf16)
                wq_sb = wpool.tile([P, NCC, GQ * D], bf16)
                wkv_sb = wpool.tile([P, NCC, 2 * D], bf16)
                cos_sb = rope_pool.tile([P, T], f32)
                sin_sb = rope_pool.tile([P, T], f32)

                # staged fat-line DMAs, completion order = consumption order;
                # first pieces quarter-sized so the PE starts by ~2us
                nc.sync.dma_start(x_sb[:, 0, 0:4, :], xq_r[:, 0, 0:4, :])
                nc.gpsimd.dma_start(wq_sb[:, 0:4, :], wq_r[:, 0:4, :])
                nc.scalar.dma_start(wkv_sb[:, 0:4, :], wkv_r[:, 0:4, :])
                nc.sync.dma_start(x_sb[:, 0, 4:8, :], xq_r[:, 0, 4:8, :])
                nc.gpsimd.dma_start(wq_sb[:, 4:16, :], wq_r[:, 4:16, :])
                nc.scalar.dma_start(wkv_sb[:, 4:16, :], wkv_r[:, 4:16, :])
                nc.sync.dma_start(x_sb[:, 0, 8:16, :], xq_r[:, 0, 8:16, :])
                nc.scalar.dma_start(x_sb[:, 2, :, :], xq_r[:, 2, :, :])
                nc.sync.dma_start(cos_sb[:], cosT)
                nc.sync.dma_start(sin_sb[:], sinT)
                nc.gpsimd.dma_start(x_sb[:, 1, :, :], xq_r[:, 1, :, :])
                nc.scalar.dma_start(x_sb[:, 3, :, :], xq_r[:, 3, :, :])
                nc.scalar.dma_start(wo_sb[:], wo_r[:])
                nc.gpsimd.dma_start(tri_sb[:], trim)
                nc.gpsimd.dma_start(ones_sb[:], onesm)
                nc.gpsimd.dma_start(rot_sb[:], rotm)
                make_identity(nc, ident)

                def rope(pt_ps, dst, use_act):
                    # dst(bf16) = pt*cos + (R pt)*sin ; pt_ps is PSUM fp32
                    raw = ptmp.tile([P, 512], f32r, name="rraw", tag="rraw")
                    if use_act:
                        nc.scalar.copy(raw[:], pt_ps[:])
                    else:
                        nc.vector.tensor_copy(raw[:], pt_ps[:])
                    rp = aux_ps.tile([P, 512], f32, name="rotp", tag="rotp")
                    nc.tensor.matmul(rp[:], rot_sb[:], raw[:], start=True, stop=True)
                    u1 = ptmp.tile([P, 512], f32, name="ru1", tag="ru1")
                    nc.vector.tensor_tensor(u1[:], raw[:], cosq(dst), MULT)
                    t2 = ptmp.tile([P, 512], f32, name="rt2", tag="rt2")
                    nc.vector.tensor_tensor(t2[:], rp[:], sinq(dst), MULT)
                    nc.vector.tensor_tensor(dst, u1[:], t2[:], ADD)

                # cos/sin slices keyed off the current chunk (set per qc)
                _trig = {}

                def cosq(_):
                    return _trig["cos"]

                def sinq(_):
                    return _trig["sin"]

                def proj_chunk(qc):
                    q0 = qc * 512
                    _trig["cos"] = cos_sb[:, q0 : q0 + 512]
                    _trig["sin"] = sin_sb[:, q0 : q0 + 512]
                    # pass 1: qp0, qp1, kp ; pass 2: qp2, qp3, vp
                    for half in range(2):
                        pa = proj_ps.tile([P, 512], f32, name="pa", tag="pp0")
                        pb = proj_ps.tile([P, 512], f32, name="pb", tag="pp1")
                        pc = proj_ps.tile([P, 512], f32, name="pc", tag="pp2")
                        h0, h1 = 2 * half, 2 * half + 1
                        for cc in range(NCC):
                            xtile = x_sb[:, qc, cc, :]
                            first, last = cc == 0, cc == NCC - 1
                            nc.tensor.matmul(
                                pa[:],
                                wq_sb[:, cc, h0 * D : (h0 + 1) * D],
                                xtile,
                                start=first,
                                stop=last,
                            )
                            nc.tensor.matmul(
                                pb[:],
                                wq_sb[:, cc, h1 * D : (h1 + 1) * D],
                                xtile,
                                start=first,
                                stop=last,
                            )
                            wv_lo = 0 if half == 0 else D
                            nc.tensor.matmul(
                                pc[:],
                                wkv_sb[:, cc, wv_lo : wv_lo + D],
                                xtile,
                                start=first,
                                stop=last,
                            )
                        rope(pa, qt_sb[h0][:, q0 : q0 + 512], True)
                        rope(pb, qt_sb[h1][:, q0 : q0 + 512], False)
                        if half == 0:
                            rope(pc, kt_sb[:, q0 : q0 + 512], True)
                        else:
                            # V: evacuate V^T then PE-transpose to [k, D]
                            vraw = ptmp.tile([P, 512], f32, name="vraw", tag="vraw")
                            nc.scalar.copy(vraw[:], pc[:])
                            for ks in range(4):
                                tp = aux_ps.tile([P, 512], f32, name="vtp", tag="rotp")
                                nc.tensor.transpose(
                                    tp[:, 0:P],
                                    vraw[:, ks * P : (ks + 1) * P],
                                    ident[:],
                                )
                                nc.vector.tensor_copy(
                                    v_sb[:, qc * 4 + ks, :], tp[:, 0:P]
                                )

                proj_chunk(0)
                attn_chunk(0)
                proj_chunk(1)
                attn_chunk(1)
                proj_chunk(2)
                proj_chunk(3)

            # ---------- phase B: attn(2..3) + all o_proj ----------
            with (
                tc.tile_pool(name="o_ps", bufs=2, space="PSUM") as o_ps,
                tc.tile_pool(name="ost", bufs=3) as ost_pool,
            ):

                def oproj_chunk(aq):
                    # qb-outer: one 4KB-line DMA per 128-row out block; the
                    # final chunk's blocks are written in halves on two
                    # queues each so the drain tail stays short.
                    for qb in range(4 * aq, 4 * aq + 4):
                        ot = ost_pool.tile([P, C_DIM], bf16, name="ot", tag="ot")
                        for ct in range(NCT):
                            op = o_ps.tile([P, 512], f32, name="op", tag="op")
                            for h in range(GQ):
                                nc.tensor.matmul(
                                    op[:],
                                    y_sb[h][:, qb * P : (qb + 1) * P],
                                    wo_sb[:, h, ct * 512 : (ct + 1) * 512],
                                    start=(h == 0),
                                    stop=(h == GQ - 1),
                                )
                            nc.vector.tensor_copy(
                                ot[:, ct * 512 : (ct + 1) * 512], op[:]
                            )
                        rows = out[qb * P : (qb + 1) * P, :]
                        if aq == NQC - 1:
                            half = C_DIM // 2
                            q1, q2 = (
                                (nc.sync, nc.gpsimd)
                                if qb % 2 == 0
                                else (nc.scalar, nc.sync)
                            )
                            q1.dma_start(rows[:, 0:half], ot[:, 0:half])
                            q2.dma_start(rows[:, half:], ot[:, half:])
                        else:
                            oq = (nc.sync, nc.gpsimd, nc.scalar)[qb % 3]
                            oq.dma_start(rows, ot[:])

                oproj_chunk(0)
                attn_chunk(2)
                oproj_chunk(1)
                attn_chunk(3)
                oproj_chunk(2)
                oproj_chunk(3)

    nc.compile()
    return nc


def make_in_maps(x, wq, wk, wv, wo, T=T_FULL):
    """Per-core input dicts for run_bass_kernel_spmd (host-packed)."""
    import ml_dtypes

    bf = ml_dtypes.bfloat16
    cosT, sinT = _rope_tables(T)
    tri = np.triu(np.ones((P, P), dtype=np.float32)).astype(bf)  # k <= q
    onesm = np.ones((P, P), dtype=np.float32).astype(bf)
    rotm = _rot_lhsT()

    def pack_x(xb):  # [T, C] -> [p, qc, cc, 512] flat
        xt = np.ascontiguousarray(xb.T)  # [C, T]
        xr = xt.reshape(16, P, 4, 512).transpose(1, 2, 0, 3)  # p qc cc t
        return np.ascontiguousarray(xr.reshape(P, -1)).astype(bf)

    xs = [pack_x(x[b]) for b in range(B)]
    in_maps = []
    for core in range(NCORES):
        b, g = core // 4, core % 4
        wqs = wq[:, 512 * g : 512 * (g + 1)]  # [C, 512]
        wks = wk[:, D * g : D * (g + 1)]  # [C, 128]
        wvs = wv[:, D * g : D * (g + 1)]
        wos = wo[512 * g : 512 * (g + 1), :]  # [512, C]
        wqp = wqs.reshape(16, P, 512).transpose(1, 0, 2).reshape(P, -1)
        wkr = wks.reshape(16, P, D).transpose(1, 0, 2)  # [p, cc, 128]
        wvr = wvs.reshape(16, P, D).transpose(1, 0, 2)
        wkvp = np.concatenate([wkr, wvr], axis=2).reshape(P, -1)
        wop = wos.reshape(GQ, P, C_DIM).transpose(1, 0, 2).reshape(P, -1)
        in_maps.append(
            {
                "xq": xs[b],
                "wqp": np.ascontiguousarray(wqp).astype(bf),
                "wkvp": np.ascontiguousarray(wkvp).astype(bf),
                "wop": np.ascontiguousarray(wop).astype(bf),
                "cosT": cosT,
                "sinT": sinT,
                "trim": tri,
                "onesm": onesm,
                "rotm": rotm,
            }
        )
    return in_maps


_NC_CACHE = {}


def _get_nc(T=T_FULL):
    if T not in _NC_CACHE:
        _NC_CACHE[T] = build_nc(T)
    return _NC_CACHE[T]


def run(inputs, trace=False):
    """Run on 8 NeuronCores. Returns (full_output, BassKernelResults)."""
    from concourse.bass_utils import run_bass_kernel_spmd

    x = np.asarray(inputs["x"], dtype=np.float32)
    in_maps = make_in_maps(
        x,
        np.asarray(inputs["wq"], dtype=np.float32),
        np.asarray(inputs["wk"], dtype=np.float32),
        np.asarray(inputs["wv"], dtype=np.float32),
        np.asarray(inputs["wo"], dtype=np.float32),
    )
    nc = _get_nc()
    res = run_bass_kernel_spmd(nc, in_maps, list(range(NCORES)), trace=trace)
    outs = res.results
    full = np.zeros((B, T_FULL, C_DIM), dtype=np.float32)
    for core in range(NCORES):
        full[core // 4] += np.asarray(outs[core]["out"], dtype=np.float32)
    return full, res


def kernel(**inputs):
    full, _ = run(inputs, trace=False)
    return full


# revision 14
# speedup vs baseline: 1.1073x; 1.1073x over previous
"""Trainium2 Bass kernel for causal self-attention with GQA + RoPE.

Model: B=2, T=2048, C=2048, H=16 query heads, H_KV=4 kv heads, D=128.

Sharding (8 NeuronCores, pure SPMD, no collectives):
  core i -> batch b = i // 4, kv-group g = i % 4
            (query heads 4g..4g+3, kv head g, all T positions of batch b).
  o_proj uses the row-slice wo[512g:512(g+1), :]; the per-core partial
  [T, C] outputs are summed on the host (the tensor-parallel all-reduce).

v4 (trace-driven): input delivery is HBM-bound (~360GB/s/core shared by all
8 cores; 13MB of packed input needs ~36us) while projections consume x+wq
at that same rate, so the tensor engine stalled ~20us early.  Fix: run the
attention chunks that need no new input BETWEEN projection chunks:

    proj(0) proj(1) attn(0) proj(2) attn(1) proj(3) | attn(2..3)+o_proj

To fit attention PSUM pools (4 banks) alongside projections, each
projection chunk is two 3-output passes (qp0/qp1/kp then qp2/qp3/vp) using
3 PSUM banks + 1 shared aux bank.  Everything else as v3: bf16 operands
(fp32 PSUM), host-packed fat-line DMAs, causal diagonal trimming with a
single [128,128] triangle mask, per-subtile S/exp pipeline, wo resident,
qb-outer o_proj with one 4KB-line output DMA per 128-row block.
"""

import math
import os

import numpy as np

os.environ.setdefault("MYCRO_LOCAL_CACHE", "1")

P = 128
D = 128
H = 16
H_KV = 4
GQ = H // H_KV  # 4 query heads per kv head (= per core)
B = 2
T_FULL = 2048
C_DIM = 2048
NCORES = 8
ROPE_BASE = 10000.0


def _rope_tables(T):
    inv_freq = 1.0 / (ROPE_BASE ** (np.arange(0, D, 2, dtype=np.float32) / D))
    t = np.arange(T, dtype=np.float32)
    freqs = np.outer(t, inv_freq)  # [T, D/2]
    emb = np.concatenate((freqs, freqs), axis=-1)  # [T, D]
    return (
        np.ascontiguousarray(np.cos(emb).T.astype(np.float32)),  # [D, T]
        np.ascontiguousarray(np.sin(emb).T.astype(np.float32)),
    )


def _rot_lhsT():
    # rotate_half(q) = R @ q with R[d, d+64] = -1 (d < 64), R[d, d-64] = +1.
    # matmul computes lhsT.T @ rhs, so pass lhsT = R^T.
    R = np.zeros((D, D), dtype=np.float32)
    half = D // 2
    R[np.arange(half), np.arange(half) + half] = -1.0
    R[np.arange(half) + half, np.arange(half)] = 1.0
    return np.ascontiguousarray(R.T)


def build_nc(T=T_FULL):
    """Build the per-core Bass/Tile program (identical across cores)."""
    from contextlib import ExitStack

    import concourse.mybir as mybir
    import concourse.tile as tile
    from concourse import bacc
    from concourse.masks import make_identity

    f32 = mybir.dt.float32
    f32r = mybir.dt.float32r
    bf16 = mybir.dt.bfloat16
    Exp = mybir.ActivationFunctionType.Exp
    MULT = mybir.AluOpType.mult
    ADD = mybir.AluOpType.add
    SCALE = 1.0 / math.sqrt(D)

    NCC = C_DIM // P  # 16 contraction chunks
    NQC = T // 512  # projection / attention q-chunks (512-wide)
    NCT = C_DIM // 512  # o_proj column tiles
    NKB = T // P  # 128-wide k subtiles

    nc = bacc.Bacc(
        "TRN2",
        target_bir_lowering=False,
        debug=False,
        num_devices=NCORES,
    )

    # host-packed inputs: per-partition-contiguous fat lines
    xq = nc.dram_tensor("xq", [P, NQC * NCC * 512], bf16, kind="ExternalInput").ap()
    wqp = nc.dram_tensor("wqp", [P, NCC * GQ * D], bf16, kind="ExternalInput").ap()
    wkvp = nc.dram_tensor("wkvp", [P, NCC * 2 * D], bf16, kind="ExternalInput").ap()
    wop = nc.dram_tensor("wop", [P, GQ * C_DIM], bf16, kind="ExternalInput").ap()
    cosT = nc.dram_tensor("cosT", [D, T], f32, kind="ExternalInput").ap()
    sinT = nc.dram_tensor("sinT", [D, T], f32, kind="ExternalInput").ap()
    trim = nc.dram_tensor("trim", [P, P], bf16, kind="ExternalInput").ap()
    onesm = nc.dram_tensor("onesm", [P, P], bf16, kind="ExternalInput").ap()
    rotm = nc.dram_tensor("rotm", [P, P], f32r, kind="ExternalInput").ap()
    out = nc.dram_tensor("out", [T, C_DIM], bf16, kind="ExternalOutput").ap()

    with tile.TileContext(nc) as tc, ExitStack() as ctx:
        const = ctx.enter_context(tc.tile_pool(name="const", bufs=1))
        acts = ctx.enter_context(tc.tile_pool(name="acts", bufs=1))

        xq_r = xq.rearrange("p (qc cc t) -> p qc cc t", qc=NQC, cc=NCC)
        wq_r = wqp.rearrange("p (cc n) -> p cc n", cc=NCC)
        wkv_r = wkvp.rearrange("p (cc n) -> p cc n", cc=NCC)
        wo_r = wop.rearrange("p (h c) -> p h c", h=GQ)

        ones_sb = const.tile([P, P], bf16)
        rot_sb = const.tile([P, P], f32r)
        ident = const.tile([P, P], f32)
        tri_sb = const.tile([P, P], bf16)

        # long-lived activations (all bf16)
        qt_sb = [acts.tile([P, T], bf16, name=f"qt{h}") for h in range(GQ)]
        kt_sb = acts.tile([P, T], bf16, name="kt")
        v_sb = acts.tile([P, NKB, D], bf16, name="vnat")
        y_sb = [acts.tile([P, T], bf16, name=f"yt{h}") for h in range(GQ)]
        wo_sb = acts.tile([P, GQ, C_DIM], bf16, name="wo_res")

        # attention pools (outermost so they span both phases)
        with (
            tc.tile_pool(name="pt_pool", bufs=6) as pt_pool,
            tc.tile_pool(name="nrm", bufs=2) as nrm_pool,
            tc.tile_pool(name="s_ps", bufs=2, space="PSUM") as s_ps,
            tc.tile_pool(name="y_ps", bufs=1, space="PSUM") as y_ps,
            tc.tile_pool(name="rs_ps", bufs=1, space="PSUM") as rs_ps,
        ):

            def attn_chunk(aq):
                q0 = aq * 512
                nks = 4 * aq + 4  # 128-wide k subtiles (incl 4 diagonal)
                dstart = nks - 4  # first diagonal subtile index
                for h in range(GQ):
                    yp = y_ps.tile([P, 512], f32, name="yp", tag="yp")
                    rp_ = rs_ps.tile([P, 512], f32, name="rsp", tag="rsp")
                    for ks in range(nks):
                        m = ks - dstart
                        a = 128 * m if m >= 0 else 0
                        sp = s_ps.tile([P, 512], f32, name="sp", tag="sp")
                        pt = pt_pool.tile([P, 512], bf16, name="ptile", tag="pt")
                        nc.tensor.matmul(
                            sp[:, a:512],
                            kt_sb[:, ks * P : (ks + 1) * P],
                            qt_sb[h][:, q0 + a : q0 + 512],
                            start=True,
                            stop=True,
                        )
                        nc.scalar.activation(
                            pt[:, a:512], sp[:, a:512], Exp, scale=SCALE
                        )
                        if m >= 0:  # diagonal: mask the leading 128 cols
                            sl = pt[:, a : a + P]
                            nc.vector.tensor_tensor(sl, sl, tri_sb[:], MULT)
                        first, last = ks == 0, ks == nks - 1
                        nc.tensor.matmul(
                            yp[:, a:512],
                            v_sb[:, ks, :],
                            pt[:, a:512],
                            start=first,
                            stop=last,
                            skip_group_check=True,
                        )
                        nc.tensor.matmul(
                            rp_[:, a:512],
                            ones_sb[:],
                            pt[:, a:512],
                            start=first,
                            stop=last,
                            skip_group_check=True,
                        )
                    # 1/rowsum on DVE (~18 bits, plenty; rowsum >= 1).
                    rinv = nrm_pool.tile([P, 512], f32, name="rinv", tag="rinv")
                    nc.vector.reciprocal_approx_fast(rinv[:], rp_[:])
                    nc.vector.tensor_tensor(
                        y_sb[h][:, q0 : q0 + 512], yp[:], rinv[:], MULT
                    )

            # ---------- phase A: projections interleaved with attn(0..1) ----
            with (
                tc.tile_pool(name="xres", bufs=1) as xres,
                tc.tile_pool(name="pwts", bufs=1) as wpool,
                tc.tile_pool(name="rope_t", bufs=1) as rope_pool,
                tc.tile_pool(name="proj_ps", bufs=1, space="PSUM") as proj_ps,
                tc.tile_pool(name="aux_ps", bufs=1, space="PSUM") as aux_ps,
                tc.tile_pool(name="ptmp", bufs=2) as ptmp,
            ):
                x_sb = xres.tile([P, NQC, NCC, 512], bf16)
                wq_sb = wpool.tile([P, NCC, GQ * D], bf16)
                wkv_sb = wpool.tile([P, NCC, 2 * D], bf16)
                cos_sb = rope_pool.tile([P, T], f32)
                sin_sb = rope_pool.tile([P, T], f32)

                # quarter-granular (0.5MB) fat-line DMAs drip-fed in
                # consumption-deadline order so the PE is never starved by a
                # multi-MB piece in flight.
                for qr in range(4):
                    lo, hi = 4 * qr, 4 * qr + 4
                    nc.sync.dma_start(x_sb[:, 0, lo:hi, :], xq_r[:, 0, lo:hi, :])
                    nc.gpsimd.dma_start(wq_sb[:, lo:hi, :], wq_r[:, lo:hi, :])
                    if qr == 0:
                        nc.scalar.dma_start(wkv_sb[:, 0:4, :], wkv_r[:, 0:4, :])
                    elif qr == 1:
                        nc.scalar.dma_start(wkv_sb[:, 4:16, :], wkv_r[:, 4:16, :])
                for qr in range(4):
                    lo, hi = 4 * qr, 4 * qr + 4
                    nc.sync.dma_start(x_sb[:, 1, lo:hi, :], xq_r[:, 1, lo:hi, :])
                nc.scalar.dma_start(cos_sb[:], cosT)
                nc.scalar.dma_start(sin_sb[:], sinT)
                nc.gpsimd.dma_start(tri_sb[:], trim)
                nc.gpsimd.dma_start(ones_sb[:], onesm)
                nc.gpsimd.dma_start(rot_sb[:], rotm)
                for qr in range(4):
                    lo, hi = 4 * qr, 4 * qr + 4
                    nc.gpsimd.dma_start(x_sb[:, 2, lo:hi, :], xq_r[:, 2, lo:hi, :])
                    nc.scalar.dma_start(x_sb[:, 3, lo:hi, :], xq_r[:, 3, lo:hi, :])
                nc.gpsimd.dma_start(wo_sb[:], wo_r[:])
                make_identity(nc, ident)

                def rope(pt_ps, dst, use_act):
                    # dst(bf16) = pt*cos + (R pt)*sin ; pt_ps is PSUM fp32
                    raw = ptmp.tile([P, 512], f32r, name="rraw", tag="rraw")
                    if use_act:
                        nc.scalar.copy(raw[:], pt_ps[:])
                    else:
                        nc.vector.tensor_copy(raw[:], pt_ps[:])
                    rp = aux_ps.tile([P, 512], f32, name="rotp", tag="rotp")
                    nc.tensor.matmul(rp[:], rot_sb[:], raw[:], start=True, stop=True)
                    u1 = ptmp.tile([P, 512], f32, name="ru1", tag="ru1")
                    nc.vector.tensor_tensor(u1[:], raw[:], cosq(dst), MULT)
                    t2 = ptmp.tile([P, 512], f32, name="rt2", tag="rt2")
                    nc.vector.tensor_tensor(t2[:], rp[:], sinq(dst), MULT)
                    nc.vector.tensor_tensor(dst, u1[:], t2[:], ADD)

                # cos/sin slices keyed off the current chunk (set per qc)
                _trig = {}

                def cosq(_):
                    return _trig["cos"]

                def sinq(_):
                    return _trig["sin"]

                def proj_chunk(qc):
                    q0 = qc * 512
                    _trig["cos"] = cos_sb[:, q0 : q0 + 512]
                    _trig["sin"] = sin_sb[:, q0 : q0 + 512]
                    # pass 1: qp0, qp1, kp ; pass 2: qp2, qp3, vp
                    for half in range(2):
                        pa = proj_ps.tile([P, 512], f32, name="pa", tag="pp0")
                        pb = proj_ps.tile([P, 512], f32, name="pb", tag="pp1")
                        pc = proj_ps.tile([P, 512], f32, name="pc", tag="pp2")
                        h0, h1 = 2 * half, 2 * half + 1
                        for cc in range(NCC):
                            xtile = x_sb[:, qc, cc, :]
                            first, last = cc == 0, cc == NCC - 1
                            nc.tensor.matmul(
                                pa[:],
                                wq_sb[:, cc, h0 * D : (h0 + 1) * D],
                                xtile,
                                start=first,
                                stop=last,
                            )
                            nc.tensor.matmul(
                                pb[:],
                                wq_sb[:, cc, h1 * D : (h1 + 1) * D],
                                xtile,
                                start=first,
                                stop=last,
                            )
                            wv_lo = 0 if half == 0 else D
                            nc.tensor.matmul(
                                pc[:],
                                wkv_sb[:, cc, wv_lo : wv_lo + D],
                                xtile,
                                start=first,
                                stop=last,
                            )
                        rope(pa, qt_sb[h0][:, q0 : q0 + 512], True)
                        rope(pb, qt_sb[h1][:, q0 : q0 + 512], False)
                        if half == 0:
                            rope(pc, kt_sb[:, q0 : q0 + 512], True)
                        else:
                            # V: evacuate V^T then PE-transpose to [k, D]
                            vraw = ptmp.tile([P, 512], f32, name="vraw", tag="vraw")
                            nc.scalar.copy(vraw[:], pc[:])
                            for ks in range(4):
                                tp = aux_ps.tile([P, 512], f32, name="vtp", tag="rotp")
                                nc.tensor.transpose(
                                    tp[:, 0:P],
                                    vraw[:, ks * P : (ks + 1) * P],
                                    ident[:],
                                )
                                nc.vector.tensor_copy(
                                    v_sb[:, qc * 4 + ks, :], tp[:, 0:P]
                                )

                proj_chunk(0)
                proj_chunk(1)
                attn_chunk(0)
                proj_chunk(2)
                attn_chunk(1)
                proj_chunk(3)

            # ---------- phase B: attn(2..3) + all o_proj ----------
            with (
                tc.tile_pool(name="o_ps", bufs=2, space="PSUM") as o_ps,
                tc.tile_pool(name="ost", bufs=3) as ost_pool,
            ):

                def oproj_chunk(aq):
                    # qb-outer: one 4KB-line DMA per 128-row out block; the
                    # final chunk's blocks are written in halves on two
                    # queues each so the drain tail stays short.
                    for qb in range(4 * aq, 4 * aq + 4):
                        ot = ost_pool.tile([P, C_DIM], bf16, name="ot", tag="ot")
                        for ct in range(NCT):
                            op = o_ps.tile([P, 512], f32, name="op", tag="op")
                            for h in range(GQ):
                                nc.tensor.matmul(
                                    op[:],
                                    y_sb[h][:, qb * P : (qb + 1) * P],
                                    wo_sb[:, h, ct * 512 : (ct + 1) * 512],
                                    start=(h == 0),
                                    stop=(h == GQ - 1),
                                )
                            nc.vector.tensor_copy(
                                ot[:, ct * 512 : (ct + 1) * 512], op[:]
                            )
                        rows = out[qb * P : (qb + 1) * P, :]
                        if aq == NQC - 1:
                            half = C_DIM // 2
                            q1, q2 = (
                                (nc.sync, nc.gpsimd)
                                if qb % 2 == 0
                                else (nc.scalar, nc.sync)
                            )
                            q1.dma_start(rows[:, 0:half], ot[:, 0:half])
                            q2.dma_start(rows[:, half:], ot[:, half:])
                        else:
                            oq = (nc.sync, nc.gpsimd, nc.scalar)[qb % 3]
                            oq.dma_start(rows, ot[:])

                oproj_chunk(0)
                attn_chunk(2)
                oproj_chunk(1)
                attn_chunk(3)
                oproj_chunk(2)
                oproj_chunk(3)

    nc.compile()
    return nc


def make_in_maps(x, wq, wk, wv, wo, T=T_FULL):
    """Per-core input dicts for run_bass_kernel_spmd (host-packed)."""
    import ml_dtypes

    bf = ml_dtypes.bfloat16
    cosT, sinT = _rope_tables(T)
    tri = np.triu(np.ones((P, P), dtype=np.float32)).astype(bf)  # k <= q
    onesm = np.ones((P, P), dtype=np.float32).astype(bf)
    rotm = _rot_lhsT()

    def pack_x(xb):  # [T, C] -> [p, qc, cc, 512] flat
        xt = np.ascontiguousarray(xb.T)  # [C, T]
        xr = xt.reshape(16, P, 4, 512).transpose(1, 2, 0, 3)  # p qc cc t
        return np.ascontiguousarray(xr.reshape(P, -1)).astype(bf)

    xs = [pack_x(x[b]) for b in range(B)]
    in_maps = []
    for core in range(NCORES):
        b, g = core // 4, core % 4
        wqs = wq[:, 512 * g : 512 * (g + 1)]  # [C, 512]
        wks = wk[:, D * g : D * (g + 1)]  # [C, 128]
        wvs = wv[:, D * g : D * (g + 1)]
        wos = wo[512 * g : 512 * (g + 1), :]  # [512, C]
        wqp = wqs.reshape(16, P, 512).transpose(1, 0, 2).reshape(P, -1)
        wkr = wks.reshape(16, P, D).transpose(1, 0, 2)  # [p, cc, 128]
        wvr = wvs.reshape(16, P, D).transpose(1, 0, 2)
        wkvp = np.concatenate([wkr, wvr], axis=2).reshape(P, -1)
        wop = wos.reshape(GQ, P, C_DIM).transpose(1, 0, 2).reshape(P, -1)
        in_maps.append(
            {
                "xq": xs[b],
                "wqp": np.ascontiguousarray(wqp).astype(bf),
                "wkvp": np.ascontiguousarray(wkvp).astype(bf),
                "wop": np.ascontiguousarray(wop).astype(bf),
                "cosT": cosT,
                "sinT": sinT,
                "trim": tri,
                "onesm": onesm,
                "rotm": rotm,
            }
        )
    return in_maps


_NC_CACHE = {}


def _get_nc(T=T_FULL):
    if T not in _NC_CACHE:
        _NC_CACHE[T] = build_nc(T)
    return _NC_CACHE[T]


def run(inputs, trace=False):
    """Run on 8 NeuronCores. Returns (full_output, BassKernelResults)."""
    from concourse.bass_utils import run_bass_kernel_spmd

    x = np.asarray(inputs["x"], dtype=np.float32)
    in_maps = make_in_maps(
        x,
        np.asarray(inputs["wq"], dtype=np.float32),
        np.asarray(inputs["wk"], dtype=np.float32),
        np.asarray(inputs["wv"], dtype=np.float32),
        np.asarray(inputs["wo"], dtype=np.float32),
    )
    nc = _get_nc()
    res = run_bass_kernel_spmd(nc, in_maps, list(range(NCORES)), trace=trace)
    outs = res.results
    full = np.zeros((B, T_FULL, C_DIM), dtype=np.float32)
    for core in range(NCORES):
        full[core // 4] += np.asarray(outs[core]["out"], dtype=np.float32)
    return full, res


def kernel(**inputs):
    full, _ = run(inputs, trace=False)
    return full
